# revision 41
# baseline (speedup 1.0000x reference)
"""MoE-routed dynamic conv kernel for Trainium2 (8 NeuronCores, SPMD).

Problem: per-sample attention (global avg pool -> 1x1 conv -> sigmoid) mixes
K=4 expert 3x3 conv kernels; each sample is convolved with its own mixed
kernel.  x: (32, 256, 56, 56), att_w: (4, 256), weight: (4, 256, 256, 3, 3).

Strategy: data parallel over batch (4 samples per core, weights replicated),
with the conv computed as 1-D Winograd F(2,3) along W (direct along H), all
tensors bf16 on the wire and in SBUF (fp32 PSUM accumulation):
  - x is zero-padded to (58, 58) on the host, cast to bf16 and split into
    even/odd column-parity planes so every device op streams step-1.
  - the K expert banks are pre-transformed on the host along kw with
    G = [[1,0,0],[.5,.5,.5],[.5,-.5,.5],[0,0,1]] -> layout (Cin, K, j, kh, Co).
  - attention (2 samples ahead): pooled sums via ACT accumulator, logits via
    a GPSIMD partition all-reduce of att_w * pooled, sigmoid on ACT.  No
    PSUM used, so the conv owns all 8 banks.
  - expert mixing (1 sample ahead): the 4 products att_k * w_k run on ACT
    as activation-copies with per-partition scale; DVE folds them with an
    in-place add chain (bf16 2x mode).
  - input transform (1 sample ahead): D_j = B^T-combo of the parity planes,
    4 DVE tensor_tensor ops per ci-block (bf16 2x).
  - GEMM: per (co-block, 14-row chunk): 4 j-planes x (3 kh x 2 ci-blocks)
    bf16 matmuls accumulate N=392 columns into bank-aligned PSUM planes
    (two 4-bank chunk tiles ping-pong).
  - eviction: one GPSIMD copy per chunk moves the 4 planes to SBUF as bf16;
    DVE collapses them (Ye = M0+M1+M2, Yo = M1-M2-M3) into parity output
    planes which DMA out as bf16; the host interleaves parities and
    upcasts to f32.

Engine-queue emission order is chosen so every engine's in-order queue sees
work in the order it becomes ready (PAR ahead of evictions on GPSIMD, mix
products ahead of nothing on ACT, D/mix ahead of collapse on DVE), keeping
the PE fed back-to-back.
"""

import sys

if "/opt/trn_rl_repo" not in sys.path:
    sys.path.insert(0, "/opt/trn_rl_repo")

import numpy as np

B_TOTAL = 32
N_CORES = 8
BPC = B_TOTAL // N_CORES  # 4
CI = 256
CO = 256
K = 4
H = W = 56
PH = 58                   # padded rows
TWP = 29                  # parity-plane cols (58/2)
TW = 28                   # output tiles per row (W/2)
NJ = 4                    # wino points
NDR = 3                   # kh taps
XF = 2 * PH * TWP         # 3364 x elems per channel (par, h, twp)
WF = NJ * NDR * CO        # 3072 wino weight elems per (k, ci): (j, dr, co)
DF = NJ * PH * TW         # 6496 D elems per channel (j, h, tw)
CHR = 14                  # oh rows per PSUM chunk
NCH = H // CHR            # 4 chunks
NC_ = CHR * TW            # 392 cols per chunk (<= 512 psum bank)
OF = H * TW               # 1568 out elems per parity per co-block

_cache = {}


def _build_nc():
    from contextlib import ExitStack

    import concourse.bacc as bacc
    import concourse.bass_isa as bass_isa
    import concourse.mybir as mybir
    import concourse.tile as tile

    f32 = mybir.dt.float32
    bf16 = mybir.dt.bfloat16
    AF = mybir.ActivationFunctionType
    ALU = mybir.AluOpType
    AX = mybir.AxisListType

    nc = bacc.Bacc("TRN2", target_bir_lowering=False, debug=False)
    x_p = nc.declare_dram_parameter("x", [BPC, CI, XF], bf16, isOutput=False)
    w_p = nc.declare_dram_parameter("w", [CI, K, WF], bf16, isOutput=False)
    aw_p = nc.declare_dram_parameter("aw", [CI, K], f32, isOutput=False)
    o_p = nc.declare_dram_parameter("out", [BPC, 2, CO, OF], bf16, isOutput=True)

    with ExitStack() as ctx:
        tc = ctx.enter_context(tile.TileContext(nc))
        pw = ctx.enter_context(tc.tile_pool(name="wpool", bufs=1))
        px = ctx.enter_context(tc.tile_pool(name="xpool", bufs=2))
        pd = ctx.enter_context(tc.tile_pool(name="dpool", bufs=2))
        pagg = ctx.enter_context(tc.tile_pool(name="aggpool", bufs=2))
        pu = ctx.enter_context(tc.tile_pool(name="mixu", bufs=3))
        put1 = ctx.enter_context(tc.tile_pool(name="mixt1", bufs=2))
        pm = ctx.enter_context(tc.tile_pool(name="mpool", bufs=3))
        psd = ctx.enter_context(tc.tile_pool(name="sdpool", bufs=3))
        py = ctx.enter_context(tc.tile_pool(name="ypool", bufs=1))
        psml = ctx.enter_context(tc.tile_pool(name="small", bufs=3))
        pdump = ctx.enter_context(tc.tile_pool(name="dump", bufs=1))
        pps = ctx.enter_context(tc.tile_pool(name="cpsum", bufs=2, space="PSUM"))

        # Resident replicated weights, loaded per (ci-block, expert) so the
        # first mixing products can start before the whole bank lands.
        # DMA-queue order interleaves with the x(0) load (emitted by the
        # pipeline below before load_weights is called).
        aw_sb = [
            pw.tile([128, K], f32, tag=f"aw{c}", name=f"aw{c}") for c in range(2)
        ]
        w_sb = [
            pw.tile([128, K * WF], bf16, tag=f"w{c}", name=f"wt{c}")
            for c in range(2)
        ]

        def load_weights():
            for c in range(2):
                nc.sync.dma_start(
                    out=aw_sb[c][:, :], in_=aw_p[c * 128 : (c + 1) * 128, :]
                )
            # k-major so sample 0's mixing chases the arrivals; c1 first
            # within each k because c1's products are the cheap DVE
            # tensor_scalar path that gates the (c1-first) first conv chunk.
            for k, c in [(0, 1), (0, 0), (1, 1), (1, 0), (2, 1), (2, 0), (3, 1), (3, 0)]:
                nc.sync.dma_start(
                    out=w_sb[c][:, k * WF : (k + 1) * WF],
                    in_=w_p[c * 128 : (c + 1) * 128, k, :],
                )

        xts = {}
        atts = {}
        aggs = {}
        dts = {}
        pools = {}

        def stage_load(b):
            xb = []
            for c in range(2):
                xt = px.tile([128, XF], bf16, tag=f"x{c}")
                nc.sync.dma_start(
                    out=xt[:, :], in_=x_p[b, c * 128 : (c + 1) * 128, :]
                )
                xb.append(xt)
            xts[b] = xb

        def stage_pool(b, head=False):
            """Pooled channel sums on ACT (accumulator), emitted right after
            the x DMA so it runs as the data lands.  head=True (sample 0)
            puts ci-block 1 on DVE instead so the two halves run in
            parallel and the sigmoid unblocks sooner."""
            xb = xts[b]
            pl = []
            for c in range(2):
                pc = psml.tile([128, 1], f32, tag=f"pooled{c}")
                if head and c == 1:
                    nc.vector.tensor_reduce(
                        pc[:, :], xb[c][:, :], axis=AX.X, op=ALU.add
                    )
                else:
                    dump = pdump.tile([128, XF], bf16, tag="pooldump")
                    nc.scalar.activation(
                        dump[:, :], xb[c][:, :], AF.Copy, accum_out=pc[:, :]
                    )
                pl.append(pc)
            pools[b] = pl

        def stage_att_reduce(b):
            """pooled -> logits via GPSIMD partition all-reduce -> sigmoid."""
            pooled = pools.pop(b)
            tka = psml.tile([128, K], f32, tag="tka")
            tkb = psml.tile([128, K], f32, tag="tkb")
            nc.vector.tensor_scalar_mul(tka[:, :], aw_sb[0][:, :], pooled[0][:, :])
            nc.vector.tensor_scalar_mul(tkb[:, :], aw_sb[1][:, :], pooled[1][:, :])
            nc.vector.tensor_tensor(tka[:, :], tka[:, :], tkb[:, :], ALU.add)
            logit = psml.tile([128, K], f32, tag="logit")
            nc.gpsimd.partition_all_reduce(
                logit[:, :], tka[:, :], 128, bass_isa.ReduceOp.add
            )
            att = psml.tile([128, K], f32, tag="att")
            nc.scalar.activation(
                att[:, :], logit[:, :], AF.Sigmoid, scale=1.0 / (H * W)
            )
            atts[b] = att

        def d_ops(xt, c, eng):
            """Emit the 4 B^T-combo ops for ci-block c of one sample."""
            x3 = xt[:, :].rearrange("p (q h t) -> p q h t", q=2, h=PH)
            xe0 = x3[:, 0, :, 0:TW]
            xe1 = x3[:, 0, :, 1 : TW + 1]
            xo0 = x3[:, 1, :, 0:TW]
            xo1 = x3[:, 1, :, 1 : TW + 1]
            dt = pd.tile([128, DF], bf16, tag=f"d{c}", name=f"dt{c}")
            d3 = dt[:, :].rearrange("p (j h t) -> p j h t", j=NJ, h=PH)
            eng.tensor_tensor(d3[:, 0], xe0, xe1, ALU.subtract)
            eng.tensor_tensor(d3[:, 1], xo0, xe1, ALU.add)
            eng.tensor_tensor(d3[:, 2], xe1, xo0, ALU.subtract)
            eng.tensor_tensor(d3[:, 3], xo0, xo1, ALU.subtract)
            return dt

        def stage_d(b):
            """Input transform: D_j from parity planes, 4 TT per ci-block."""
            xb = xts.pop(b)
            dts[b] = [d_ops(xb[c], c, nc.vector) for c in range(2)]

        def stage_mix(b, head=False):
            """agg_c = sum_k att_k * w_k.

            Steady state (head=False): ci-block 0's four products on ACT;
            ci-block 1 takes k0/k1 as DVE tensor_scalar (4x mode), k2/k3 on
            ACT.  DVE folds with in-place adds (bf16 2x) in readiness order.

            head=True (sample 0): the per-k weight DMAs gate everything, so
            only c0/k0..k2 go to ACT; the DMA-critical tail (k3c0 and all of
            c1) runs as DVE tensor_scalar, which is cheap enough to chase
            each arriving weight slice.
            """
            att = atts.pop(b)
            TT = nc.vector.tensor_tensor
            TS = nc.vector.tensor_scalar_mul

            def wslice(c, k):
                return w_sb[c][:, k * WF : (k + 1) * WF]

            def act_prod(dst, c, k):
                nc.scalar.activation(
                    dst[:, :], wslice(c, k), AF.Copy, scale=att[:, k : k + 1]
                )

            ag0 = pagg.tile([128, WF], bf16, tag="agg0")
            ag1 = pagg.tile([128, WF], bf16, tag="agg1")
            if head:
                u1c0 = pu.tile([128, WF], bf16, tag="u")
                u2c0 = pu.tile([128, WF], bf16, tag="u")
                act_prod(ag0, 0, 0)
                act_prod(u1c0, 0, 1)
                act_prod(u2c0, 0, 2)
                ta = put1.tile([128, WF], bf16, tag="t1")
                tb = put1.tile([128, WF], bf16, tag="t1")
                tc_ = put1.tile([128, WF], bf16, tag="t1")
                td = put1.tile([128, WF], bf16, tag="t1")
                TS(ag1[:, :], wslice(1, 0), att[:, 0:1])
                TS(ta[:, :], wslice(1, 1), att[:, 1:2])
                TT(ag1[:, :], ag1[:, :], ta[:, :], ALU.add)
                TT(ag0[:, :], ag0[:, :], u1c0[:, :], ALU.add)
                TS(tb[:, :], wslice(1, 2), att[:, 2:3])
                TT(ag1[:, :], ag1[:, :], tb[:, :], ALU.add)
                TS(td[:, :], wslice(1, 3), att[:, 3:4])
                TT(ag1[:, :], ag1[:, :], td[:, :], ALU.add)
                TT(ag0[:, :], ag0[:, :], u2c0[:, :], ALU.add)
                TS(tc_[:, :], wslice(0, 3), att[:, 3:4])
                TT(ag0[:, :], ag0[:, :], tc_[:, :], ALU.add)
            else:
                u1c0 = pu.tile([128, WF], bf16, tag="u")
                u2c0 = pu.tile([128, WF], bf16, tag="u")
                u2c1 = pu.tile([128, WF], bf16, tag="u")
                u3c0 = pu.tile([128, WF], bf16, tag="u")
                u3c1 = pu.tile([128, WF], bf16, tag="u")
                act_prod(ag0, 0, 0)
                act_prod(u1c0, 0, 1)
                act_prod(u2c0, 0, 2)
                act_prod(u2c1, 1, 2)
                act_prod(u3c0, 0, 3)
                act_prod(u3c1, 1, 3)
                t1 = put1.tile([128, WF], bf16, tag="t1")
                TS(ag1[:, :], wslice(1, 0), att[:, 0:1])
                TS(t1[:, :], wslice(1, 1), att[:, 1:2])
                TT(ag1[:, :], ag1[:, :], t1[:, :], ALU.add)
                TT(ag0[:, :], ag0[:, :], u1c0[:, :], ALU.add)
                TT(ag0[:, :], ag0[:, :], u2c0[:, :], ALU.add)
                TT(ag1[:, :], ag1[:, :], u2c1[:, :], ALU.add)
                TT(ag0[:, :], ag0[:, :], u3c0[:, :], ALU.add)
                TT(ag1[:, :], ag1[:, :], u3c1[:, :], ALU.add)
            aggs[b] = [ag0, ag1]

        def stage_conv(b):
            """GEMM chunks with interleaved eviction (GPSIMD) and collapse
            (DVE), then the output DMAs."""
            db = dts.pop(b)
            ab = aggs.pop(b)
            d3s = [
                dt[:, :].rearrange("p (j h t) -> p j h t", j=NJ, h=PH) for dt in db
            ]
            for cb in range(2):
                yt = py.tile([128, 2 * OF], bf16, tag="y")
                y4 = yt[:, :].rearrange("p (q h t) -> p q h t", q=2, h=H)
                for ch in range(NCH):
                    r0 = ch * CHR
                    ps = pps.tile([128, NJ * 512], f32, tag="convps")
                    for j in range(NJ):
                        out3 = ps[:, j * 512 : j * 512 + NC_].rearrange(
                            "p (h t) -> p h t", h=CHR
                        )
                        i = 0
                        for c in (1, 0):  # ag1 lands first in the preamble
                            for dr in range(NDR):
                                base = (j * NDR + dr) * CO + cb * 128
                                nc.tensor.matmul(
                                    out3[:, :, :],
                                    lhsT=ab[c][:, base : base + 128],
                                    rhs=d3s[c][:, j, r0 + dr : r0 + dr + CHR, :],
                                    start=(i == 0),
                                    stop=(i == 2 * NDR - 1),
                                )
                                i += 1
                    mt = pm.tile([128, NJ * NC_], bf16, tag="m")
                    nc.gpsimd.tensor_copy(
                        mt[:, :].rearrange("p (j n) -> p j n", j=NJ),
                        ps[:, :].rearrange("p (j n) -> p j n", j=NJ)[:, :, 0:NC_],
                    )
                    # collapse this chunk on DVE
                    m3 = mt[:, :].rearrange("p (j h t) -> p j h t", j=NJ, h=CHR)
                    s = psd.tile([128, NC_], bf16, tag="s")
                    d = psd.tile([128, NC_], bf16, tag="d")
                    s3 = s[:, :].rearrange("p (h t) -> p h t", h=CHR)
                    d3 = d[:, :].rearrange("p (h t) -> p h t", h=CHR)
                    # s/d on GPSIMD (right after its eviction) frees DVE slack
                    nc.gpsimd.tensor_tensor(s3, m3[:, 1], m3[:, 2], ALU.add)
                    nc.gpsimd.tensor_tensor(d3, m3[:, 1], m3[:, 2], ALU.subtract)
                    nc.vector.tensor_tensor(
                        y4[:, 0, r0 : r0 + CHR, :], m3[:, 0], s3, ALU.add
                    )
                    nc.vector.tensor_tensor(
                        y4[:, 1, r0 : r0 + CHR, :], d3, m3[:, 3], ALU.subtract
                    )
                    if ch % 2 == 1:
                        # flush the finished half so the tail DMA is short
                        hf = ch // 2
                        half = OF // 2
                        for par in range(2):
                            nc.sync.dma_start(
                                out=o_p[
                                    b,
                                    par,
                                    cb * 128 : (cb + 1) * 128,
                                    hf * half : (hf + 1) * half,
                                ],
                                in_=yt[:, :].rearrange("p (q f) -> p q f", q=2)[
                                    :, par, hf * half : (hf + 1) * half
                                ],
                            )

        # ---- software pipeline ----
        # Preamble (sample 0): x(0) heads the DMA queue, then the per-k
        # weight loads.  A dummy sigmoid at t=0 preloads the act-function
        # table off the critical path; pooling is split ACT/DVE; the D
        # transform is split DVE/GPSIMD; warm-up matmuls gated on an early
        # weight slice keep the PE p-state at full clock by the time the
        # first real matmul issues.
        dsig = psml.tile([128, 2], f32, tag="dsig")
        nc.vector.memset(dsig[:, :], 0.0)
        nc.scalar.activation(dsig[:, :], dsig[:, :], AF.Sigmoid)
        stage_load(0)
        load_weights()
        # pooled: c0 on ACT (accumulator), c1 on DVE
        pc0 = psml.tile([128, 1], f32, tag="pooled0")
        dump = pdump.tile([128, XF], bf16, tag="pooldump")
        nc.scalar.activation(dump[:, :], xts[0][0][:, :], AF.Copy, accum_out=pc0[:, :])
        dt_c0 = d_ops(xts[0][0], 0, nc.vector)
        pc1 = psml.tile([128, 1], f32, tag="pooled1")
        nc.vector.tensor_reduce(pc1[:, :], xts[0][1][:, :], axis=AX.X, op=ALU.add)
        pools[0] = [pc0, pc1]
        stage_att_reduce(0)
        dt_c1 = d_ops(xts[0][1], 1, nc.gpsimd)
        dts[0] = [dt_c0, dt_c1]
        del xts[0]
        stage_mix(0, head=True)
        stage_load(1)
        stage_pool(1)
        stage_att_reduce(1)
        for b in range(BPC):
            # Window b queues: DVE: D(b+1), mix(b+1) TS+folds, collapse(b);
            # ACT: products(b+1), pooled(b+2), sigmoid(b+2) (emitted after
            # conv so GPSIMD's PAR(b+2) queues behind the evictions(b));
            # GPSIMD: evictions(b), then PAR(b+2).
            if b + 1 < BPC:
                stage_d(b + 1)
                stage_mix(b + 1)
            if b + 2 < BPC:
                stage_load(b + 2)
                stage_pool(b + 2)
            stage_conv(b)
            if b + 2 < BPC:
                stage_att_reduce(b + 2)

    nc.compile()
    return nc


def _get_nc():
    if "nc" not in _cache:
        _cache["nc"] = _build_nc()
    return _cache["nc"]


def _make_in_maps(x, att_w, weight):
    from ml_dtypes import bfloat16

    x = np.asarray(x, dtype=np.float32)
    att_w = np.asarray(att_w, dtype=np.float32)
    weight = np.asarray(weight, dtype=np.float32)
    # pad to (58, 58), split w-parity, cast bf16: (B, CI, 2, 58, 29)
    xp = np.pad(x, ((0, 0), (0, 0), (1, 1), (1, 1)))
    xh = np.empty((B_TOTAL, CI, 2, PH, TWP), dtype=bfloat16)
    xh[:, :, 0] = xp[:, :, :, 0::2]
    xh[:, :, 1] = xp[:, :, :, 1::2]
    xh = xh.reshape(B_TOTAL, CI, XF)
    # wino expert banks: (K, Cout, Cin, kh, kw) -> (Cin, K, j, kh, Cout)
    G = np.array(
        [[1, 0, 0], [0.5, 0.5, 0.5], [0.5, -0.5, 0.5], [0, 0, 1]], np.float32
    )
    wj = np.einsum("jd,koihd->ikjho", G, weight)
    wj = np.ascontiguousarray(wj).astype(bfloat16).reshape(CI, K, WF)
    awt = np.ascontiguousarray(att_w.T)  # (CI, K) f32
    return [
        {
            "x": np.ascontiguousarray(xh[i * BPC : (i + 1) * BPC]),
            "w": wj,
            "aw": awt,
        }
        for i in range(N_CORES)
    ]


def _run(x, att_w, weight, trace=False, **spmd_kwargs):
    from concourse.bass_utils import run_bass_kernel_spmd

    nc = _get_nc()
    in_maps = _make_in_maps(x, att_w, weight)
    res = run_bass_kernel_spmd(
        nc, in_maps, list(range(N_CORES)), trace=trace, **spmd_kwargs
    )
    o = np.concatenate([r["out"] for r in res.results], axis=0)
    # (B, 2, CO, H*TW) bf16 -> interleave parities, upcast
    o = o.reshape(B_TOTAL, 2, CO, H, TW).astype(np.float32)
    out = np.empty((B_TOTAL, CO, H, W), dtype=np.float32)
    out[:, :, :, 0::2] = o[:, 0]
    out[:, :, :, 1::2] = o[:, 1]
    return out, res


def kernel(x, att_w, weight):
    out, _ = _run(x, att_w, weight)
    return out


# revision 42
# speedup vs baseline: 1.0042x; 1.0042x over previous
"""MoE-routed dynamic conv kernel for Trainium2 (8 NeuronCores, SPMD).

Problem: per-sample attention (global avg pool -> 1x1 conv -> sigmoid) mixes
K=4 expert 3x3 conv kernels; each sample is convolved with its own mixed
kernel.  x: (32, 256, 56, 56), att_w: (4, 256), weight: (4, 256, 256, 3, 3).

Strategy: data parallel over batch (4 samples per core, weights replicated),
with the conv computed as 1-D Winograd F(2,3) along W (direct along H), all
tensors bf16 on the wire and in SBUF (fp32 PSUM accumulation):
  - x is zero-padded to (58, 58) on the host, cast to bf16 and split into
    even/odd column-parity planes so every device op streams step-1.
  - the K expert banks are pre-transformed on the host along kw with
    G = [[1,0,0],[.5,.5,.5],[.5,-.5,.5],[0,0,1]] -> layout (Cin, K, j, kh, Co).
  - attention (2 samples ahead): pooled sums via ACT accumulator, logits via
    a GPSIMD partition all-reduce of att_w * pooled, sigmoid on ACT.  No
    PSUM used, so the conv owns all 8 banks.
  - expert mixing (1 sample ahead): the 4 products att_k * w_k run on ACT
    as activation-copies with per-partition scale; DVE folds them with an
    in-place add chain (bf16 2x mode).
  - input transform (1 sample ahead): D_j = B^T-combo of the parity planes,
    4 DVE tensor_tensor ops per ci-block (bf16 2x).
  - GEMM: per (co-block, 14-row chunk): 4 j-planes x (3 kh x 2 ci-blocks)
    bf16 matmuls accumulate N=392 columns into bank-aligned PSUM planes
    (two 4-bank chunk tiles ping-pong).
  - eviction: one GPSIMD copy per chunk moves the 4 planes to SBUF as bf16;
    DVE collapses them (Ye = M0+M1+M2, Yo = M1-M2-M3) into parity output
    planes which DMA out as bf16; the host interleaves parities and
    upcasts to f32.

Engine-queue emission order is chosen so every engine's in-order queue sees
work in the order it becomes ready (PAR ahead of evictions on GPSIMD, mix
products ahead of nothing on ACT, D/mix ahead of collapse on DVE), keeping
the PE fed back-to-back.
"""

import sys

if "/opt/trn_rl_repo" not in sys.path:
    sys.path.insert(0, "/opt/trn_rl_repo")

import numpy as np

B_TOTAL = 32
N_CORES = 8
BPC = B_TOTAL // N_CORES  # 4
CI = 256
CO = 256
K = 4
H = W = 56
PH = 58                   # padded rows
TWP = 29                  # parity-plane cols (58/2)
TW = 28                   # output tiles per row (W/2)
NJ = 4                    # wino points
NDR = 3                   # kh taps
XF = 2 * PH * TWP         # 3364 x elems per channel (par, h, twp)
WF = NJ * NDR * CO        # 3072 wino weight elems per (k, ci): (j, dr, co)
DF = NJ * PH * TW         # 6496 D elems per channel (j, h, tw)
CHR = 14                  # oh rows per PSUM chunk
NCH = H // CHR            # 4 chunks
NC_ = CHR * TW            # 392 cols per chunk (<= 512 psum bank)
OF = H * TW               # 1568 out elems per parity per co-block

_cache = {}


def _build_nc():
    from contextlib import ExitStack

    import concourse.bacc as bacc
    import concourse.bass_isa as bass_isa
    import concourse.mybir as mybir
    import concourse.tile as tile

    f32 = mybir.dt.float32
    bf16 = mybir.dt.bfloat16
    AF = mybir.ActivationFunctionType
    ALU = mybir.AluOpType
    AX = mybir.AxisListType

    nc = bacc.Bacc("TRN2", target_bir_lowering=False, debug=False)
    x_p = nc.declare_dram_parameter("x", [BPC, CI, XF], bf16, isOutput=False)
    w_p = nc.declare_dram_parameter("w", [CI, K, WF], bf16, isOutput=False)
    aw_p = nc.declare_dram_parameter("aw", [CI, K], f32, isOutput=False)
    o_p = nc.declare_dram_parameter("out", [BPC, 2, CO, OF], bf16, isOutput=True)

    with ExitStack() as ctx:
        tc = ctx.enter_context(tile.TileContext(nc))
        pw = ctx.enter_context(tc.tile_pool(name="wpool", bufs=1))
        px = ctx.enter_context(tc.tile_pool(name="xpool", bufs=2))
        pd = ctx.enter_context(tc.tile_pool(name="dpool", bufs=2))
        pagg = ctx.enter_context(tc.tile_pool(name="aggpool", bufs=2))
        pu = ctx.enter_context(tc.tile_pool(name="mixu", bufs=3))
        put1 = ctx.enter_context(tc.tile_pool(name="mixt1", bufs=2))
        pm = ctx.enter_context(tc.tile_pool(name="mpool", bufs=3))
        psd = ctx.enter_context(tc.tile_pool(name="sdpool", bufs=3))
        py = ctx.enter_context(tc.tile_pool(name="ypool", bufs=1))
        psml = ctx.enter_context(tc.tile_pool(name="small", bufs=3))
        pdump = ctx.enter_context(tc.tile_pool(name="dump", bufs=1))
        pps = ctx.enter_context(tc.tile_pool(name="cpsum", bufs=2, space="PSUM"))

        # Resident replicated weights, loaded per (ci-block, expert) so the
        # first mixing products can start before the whole bank lands.
        # DMA-queue order interleaves with the x(0) load (emitted by the
        # pipeline below before load_weights is called).
        aw_sb = [
            pw.tile([128, K], f32, tag=f"aw{c}", name=f"aw{c}") for c in range(2)
        ]
        w_sb = [
            pw.tile([128, K * WF], bf16, tag=f"w{c}", name=f"wt{c}")
            for c in range(2)
        ]

        def load_weights():
            for c in range(2):
                nc.sync.dma_start(
                    out=aw_sb[c][:, :], in_=aw_p[c * 128 : (c + 1) * 128, :]
                )
            # k-major so sample 0's mixing chases the arrivals; c1 first
            # within each k because c1's products are the cheap DVE
            # tensor_scalar path that gates the (c1-first) first conv chunk.
            for k, c in [(0, 1), (0, 0), (1, 1), (1, 0), (2, 1), (2, 0), (3, 1), (3, 0)]:
                nc.sync.dma_start(
                    out=w_sb[c][:, k * WF : (k + 1) * WF],
                    in_=w_p[c * 128 : (c + 1) * 128, k, :],
                )

        xts = {}
        atts = {}
        aggs = {}
        dts = {}
        pools = {}

        def stage_load(b):
            xb = []
            for c in range(2):
                xt = px.tile([128, XF], bf16, tag=f"x{c}")
                nc.sync.dma_start(
                    out=xt[:, :], in_=x_p[b, c * 128 : (c + 1) * 128, :]
                )
                xb.append(xt)
            xts[b] = xb

        def stage_pool(b, head=False):
            """Pooled channel sums on ACT (accumulator), emitted right after
            the x DMA so it runs as the data lands.  head=True (sample 0)
            puts ci-block 1 on DVE instead so the two halves run in
            parallel and the sigmoid unblocks sooner."""
            xb = xts[b]
            pl = []
            for c in range(2):
                pc = psml.tile([128, 1], f32, tag=f"pooled{c}")
                if head and c == 1:
                    nc.vector.tensor_reduce(
                        pc[:, :], xb[c][:, :], axis=AX.X, op=ALU.add
                    )
                else:
                    dump = pdump.tile([128, XF], bf16, tag="pooldump")
                    nc.scalar.activation(
                        dump[:, :], xb[c][:, :], AF.Copy, accum_out=pc[:, :]
                    )
                pl.append(pc)
            pools[b] = pl

        def stage_att_reduce(b):
            """pooled -> logits via GPSIMD partition all-reduce -> sigmoid."""
            pooled = pools.pop(b)
            tka = psml.tile([128, K], f32, tag="tka")
            tkb = psml.tile([128, K], f32, tag="tkb")
            nc.vector.tensor_scalar_mul(tka[:, :], aw_sb[0][:, :], pooled[0][:, :])
            nc.vector.tensor_scalar_mul(tkb[:, :], aw_sb[1][:, :], pooled[1][:, :])
            nc.vector.tensor_tensor(tka[:, :], tka[:, :], tkb[:, :], ALU.add)
            logit = psml.tile([128, K], f32, tag="logit")
            nc.gpsimd.partition_all_reduce(
                logit[:, :], tka[:, :], 128, bass_isa.ReduceOp.add
            )
            att = psml.tile([128, K], f32, tag="att")
            nc.scalar.activation(
                att[:, :], logit[:, :], AF.Sigmoid, scale=1.0 / (H * W)
            )
            atts[b] = att

        def d_ops(xt, c, eng):
            """Emit the 4 B^T-combo ops for ci-block c of one sample."""
            x3 = xt[:, :].rearrange("p (q h t) -> p q h t", q=2, h=PH)
            xe0 = x3[:, 0, :, 0:TW]
            xe1 = x3[:, 0, :, 1 : TW + 1]
            xo0 = x3[:, 1, :, 0:TW]
            xo1 = x3[:, 1, :, 1 : TW + 1]
            dt = pd.tile([128, DF], bf16, tag=f"d{c}", name=f"dt{c}")
            d3 = dt[:, :].rearrange("p (j h t) -> p j h t", j=NJ, h=PH)
            eng.tensor_tensor(d3[:, 0], xe0, xe1, ALU.subtract)
            eng.tensor_tensor(d3[:, 1], xo0, xe1, ALU.add)
            eng.tensor_tensor(d3[:, 2], xe1, xo0, ALU.subtract)
            eng.tensor_tensor(d3[:, 3], xo0, xo1, ALU.subtract)
            return dt

        def stage_d(b):
            """Input transform: D_j from parity planes, 4 TT per ci-block."""
            xb = xts.pop(b)
            dts[b] = [d_ops(xb[c], c, nc.vector) for c in range(2)]

        def stage_mix(b, head=False):
            """agg_c = sum_k att_k * w_k.

            Steady state (head=False): ci-block 0's four products on ACT;
            ci-block 1 takes k0/k1 as DVE tensor_scalar (4x mode), k2/k3 on
            ACT.  DVE folds with in-place adds (bf16 2x) in readiness order.

            head=True (sample 0): the per-k weight DMAs gate everything, so
            only c0/k0..k2 go to ACT; the DMA-critical tail (k3c0 and all of
            c1) runs as DVE tensor_scalar, which is cheap enough to chase
            each arriving weight slice.
            """
            att = atts.pop(b)
            TT = nc.vector.tensor_tensor
            TS = nc.vector.tensor_scalar_mul

            def wslice(c, k):
                return w_sb[c][:, k * WF : (k + 1) * WF]

            def act_prod(dst, c, k):
                nc.scalar.activation(
                    dst[:, :], wslice(c, k), AF.Copy, scale=att[:, k : k + 1]
                )

            ag0 = pagg.tile([128, WF], bf16, tag="agg0")
            ag1 = pagg.tile([128, WF], bf16, tag="agg1")
            if head:
                u1c0 = pu.tile([128, WF], bf16, tag="u")
                u2c0 = pu.tile([128, WF], bf16, tag="u")
                act_prod(ag0, 0, 0)
                act_prod(u1c0, 0, 1)
                act_prod(u2c0, 0, 2)
                ta = put1.tile([128, WF], bf16, tag="t1")
                tb = put1.tile([128, WF], bf16, tag="t1")
                tc_ = put1.tile([128, WF], bf16, tag="t1")
                td = put1.tile([128, WF], bf16, tag="t1")
                TS(ag1[:, :], wslice(1, 0), att[:, 0:1])
                TS(ta[:, :], wslice(1, 1), att[:, 1:2])
                TT(ag1[:, :], ag1[:, :], ta[:, :], ALU.add)
                TT(ag0[:, :], ag0[:, :], u1c0[:, :], ALU.add)
                TS(tb[:, :], wslice(1, 2), att[:, 2:3])
                TT(ag1[:, :], ag1[:, :], tb[:, :], ALU.add)
                TS(td[:, :], wslice(1, 3), att[:, 3:4])
                TT(ag1[:, :], ag1[:, :], td[:, :], ALU.add)
                TT(ag0[:, :], ag0[:, :], u2c0[:, :], ALU.add)
                TS(tc_[:, :], wslice(0, 3), att[:, 3:4])
                TT(ag0[:, :], ag0[:, :], tc_[:, :], ALU.add)
            else:
                u1c0 = pu.tile([128, WF], bf16, tag="u")
                u2c0 = pu.tile([128, WF], bf16, tag="u")
                u2c1 = pu.tile([128, WF], bf16, tag="u")
                u3c0 = pu.tile([128, WF], bf16, tag="u")
                u3c1 = pu.tile([128, WF], bf16, tag="u")
                act_prod(ag0, 0, 0)
                act_prod(u1c0, 0, 1)
                act_prod(u2c0, 0, 2)
                act_prod(u2c1, 1, 2)
                act_prod(u3c0, 0, 3)
                act_prod(u3c1, 1, 3)
                t1 = put1.tile([128, WF], bf16, tag="t1")
                TS(ag1[:, :], wslice(1, 0), att[:, 0:1])
                TS(t1[:, :], wslice(1, 1), att[:, 1:2])
                TT(ag1[:, :], ag1[:, :], t1[:, :], ALU.add)
                TT(ag0[:, :], ag0[:, :], u1c0[:, :], ALU.add)
                TT(ag0[:, :], ag0[:, :], u2c0[:, :], ALU.add)
                TT(ag1[:, :], ag1[:, :], u2c1[:, :], ALU.add)
                TT(ag0[:, :], ag0[:, :], u3c0[:, :], ALU.add)
                TT(ag1[:, :], ag1[:, :], u3c1[:, :], ALU.add)
            aggs[b] = [ag0, ag1]

        def stage_conv(b):
            """GEMM chunks with interleaved eviction (GPSIMD) and collapse
            (DVE), then the output DMAs."""
            db = dts.pop(b)
            ab = aggs.pop(b)
            d3s = [
                dt[:, :].rearrange("p (j h t) -> p j h t", j=NJ, h=PH) for dt in db
            ]
            for cb in range(2):
                yt = py.tile([128, 2 * OF], bf16, tag="y")
                y4 = yt[:, :].rearrange("p (q h t) -> p q h t", q=2, h=H)
                for ch in range(NCH):
                    r0 = ch * CHR
                    ps = pps.tile([128, NJ * 512], f32, tag="convps")
                    # two passes over the j-groups (c1 then c0) so the PE
                    # has 12 matmuls of runway before agg(c0) is needed
                    # (matters for the first sample, harmless after)
                    for c in (1, 0):
                        for j in range(NJ):
                            out3 = ps[:, j * 512 : j * 512 + NC_].rearrange(
                                "p (h t) -> p h t", h=CHR
                            )
                            for dr in range(NDR):
                                base = (j * NDR + dr) * CO + cb * 128
                                nc.tensor.matmul(
                                    out3[:, :, :],
                                    lhsT=ab[c][:, base : base + 128],
                                    rhs=d3s[c][:, j, r0 + dr : r0 + dr + CHR, :],
                                    start=(c == 1 and dr == 0),
                                    stop=(c == 0 and dr == NDR - 1),
                                )
                    mt = pm.tile([128, NJ * NC_], bf16, tag="m")
                    nc.gpsimd.tensor_copy(
                        mt[:, :].rearrange("p (j n) -> p j n", j=NJ),
                        ps[:, :].rearrange("p (j n) -> p j n", j=NJ)[:, :, 0:NC_],
                    )
                    # collapse this chunk on DVE
                    m3 = mt[:, :].rearrange("p (j h t) -> p j h t", j=NJ, h=CHR)
                    s = psd.tile([128, NC_], bf16, tag="s")
                    d = psd.tile([128, NC_], bf16, tag="d")
                    s3 = s[:, :].rearrange("p (h t) -> p h t", h=CHR)
                    d3 = d[:, :].rearrange("p (h t) -> p h t", h=CHR)
                    # s/d on GPSIMD (right after its eviction) frees DVE slack
                    nc.gpsimd.tensor_tensor(s3, m3[:, 1], m3[:, 2], ALU.add)
                    nc.gpsimd.tensor_tensor(d3, m3[:, 1], m3[:, 2], ALU.subtract)
                    nc.vector.tensor_tensor(
                        y4[:, 0, r0 : r0 + CHR, :], m3[:, 0], s3, ALU.add
                    )
                    nc.vector.tensor_tensor(
                        y4[:, 1, r0 : r0 + CHR, :], d3, m3[:, 3], ALU.subtract
                    )
                    if ch % 2 == 1:
                        # flush the finished half so the tail DMA is short
                        hf = ch // 2
                        half = OF // 2
                        for par in range(2):
                            nc.sync.dma_start(
                                out=o_p[
                                    b,
                                    par,
                                    cb * 128 : (cb + 1) * 128,
                                    hf * half : (hf + 1) * half,
                                ],
                                in_=yt[:, :].rearrange("p (q f) -> p q f", q=2)[
                                    :, par, hf * half : (hf + 1) * half
                                ],
                            )

        # ---- software pipeline ----
        # Preamble (sample 0): x(0) heads the DMA queue, then the per-k
        # weight loads.  A dummy sigmoid at t=0 preloads the act-function
        # table off the critical path; pooling is split ACT/DVE; the D
        # transform is split DVE/GPSIMD; warm-up matmuls gated on an early
        # weight slice keep the PE p-state at full clock by the time the
        # first real matmul issues.
        dsig = psml.tile([128, 2], f32, tag="dsig")
        nc.vector.memset(dsig[:, :], 0.0)
        nc.scalar.activation(dsig[:, :], dsig[:, :], AF.Sigmoid)
        stage_load(0)
        load_weights()
        # pooled: c0 on ACT (accumulator), c1 on DVE
        pc0 = psml.tile([128, 1], f32, tag="pooled0")
        dump = pdump.tile([128, XF], bf16, tag="pooldump")
        nc.scalar.activation(dump[:, :], xts[0][0][:, :], AF.Copy, accum_out=pc0[:, :])
        dt_c0 = d_ops(xts[0][0], 0, nc.vector)
        pc1 = psml.tile([128, 1], f32, tag="pooled1")
        nc.vector.tensor_reduce(pc1[:, :], xts[0][1][:, :], axis=AX.X, op=ALU.add)
        pools[0] = [pc0, pc1]
        stage_att_reduce(0)
        dt_c1 = d_ops(xts[0][1], 1, nc.gpsimd)
        dts[0] = [dt_c0, dt_c1]
        del xts[0]
        stage_mix(0, head=True)
        stage_load(1)
        stage_pool(1)
        stage_att_reduce(1)
        for b in range(BPC):
            # Window b queues: DVE: D(b+1), mix(b+1) TS+folds, collapse(b);
            # ACT: products(b+1), pooled(b+2), sigmoid(b+2) (emitted after
            # conv so GPSIMD's PAR(b+2) queues behind the evictions(b));
            # GPSIMD: evictions(b), then PAR(b+2).
            if b + 1 < BPC:
                stage_d(b + 1)
                stage_mix(b + 1)
            if b + 2 < BPC:
                stage_load(b + 2)
                stage_pool(b + 2)
            stage_conv(b)
            if b + 2 < BPC:
                stage_att_reduce(b + 2)

    nc.compile()
    return nc


def _get_nc():
    if "nc" not in _cache:
        _cache["nc"] = _build_nc()
    return _cache["nc"]


def _make_in_maps(x, att_w, weight):
    from ml_dtypes import bfloat16

    x = np.asarray(x, dtype=np.float32)
    att_w = np.asarray(att_w, dtype=np.float32)
    weight = np.asarray(weight, dtype=np.float32)
    # pad to (58, 58), split w-parity, cast bf16: (B, CI, 2, 58, 29)
    xp = np.pad(x, ((0, 0), (0, 0), (1, 1), (1, 1)))
    xh = np.empty((B_TOTAL, CI, 2, PH, TWP), dtype=bfloat16)
    xh[:, :, 0] = xp[:, :, :, 0::2]
    xh[:, :, 1] = xp[:, :, :, 1::2]
    xh = xh.reshape(B_TOTAL, CI, XF)
    # wino expert banks: (K, Cout, Cin, kh, kw) -> (Cin, K, j, kh, Cout)
    G = np.array(
        [[1, 0, 0], [0.5, 0.5, 0.5], [0.5, -0.5, 0.5], [0, 0, 1]], np.float32
    )
    wj = np.einsum("jd,koihd->ikjho", G, weight)
    wj = np.ascontiguousarray(wj).astype(bfloat16).reshape(CI, K, WF)
    awt = np.ascontiguousarray(att_w.T)  # (CI, K) f32
    return [
        {
            "x": np.ascontiguousarray(xh[i * BPC : (i + 1) * BPC]),
            "w": wj,
            "aw": awt,
        }
        for i in range(N_CORES)
    ]


def _run(x, att_w, weight, trace=False, **spmd_kwargs):
    from concourse.bass_utils import run_bass_kernel_spmd

    nc = _get_nc()
    in_maps = _make_in_maps(x, att_w, weight)
    res = run_bass_kernel_spmd(
        nc, in_maps, list(range(N_CORES)), trace=trace, **spmd_kwargs
    )
    o = np.concatenate([r["out"] for r in res.results], axis=0)
    # (B, 2, CO, H*TW) bf16 -> interleave parities, upcast
    o = o.reshape(B_TOTAL, 2, CO, H, TW).astype(np.float32)
    out = np.empty((B_TOTAL, CO, H, W), dtype=np.float32)
    out[:, :, :, 0::2] = o[:, 0]
    out[:, :, :, 1::2] = o[:, 1]
    return out, res


def kernel(x, att_w, weight):
    out, _ = _run(x, att_w, weight)
    return out


# revision 43
# speedup vs baseline: 1.0185x; 1.0142x over previous
"""MoE-routed dynamic conv kernel for Trainium2 (8 NeuronCores, SPMD).

Problem: per-sample attention (global avg pool -> 1x1 conv -> sigmoid) mixes
K=4 expert 3x3 conv kernels; each sample is convolved with its own mixed
kernel.  x: (32, 256, 56, 56), att_w: (4, 256), weight: (4, 256, 256, 3, 3).

Strategy: data parallel over batch (4 samples per core, weights replicated),
with the conv computed as 1-D Winograd F(2,3) along W (direct along H), all
tensors bf16 on the wire and in SBUF (fp32 PSUM accumulation):
  - x is zero-padded to (58, 58) on the host, cast to bf16 and split into
    even/odd column-parity planes so every device op streams step-1.
  - the K expert banks are pre-transformed on the host along kw with
    G = [[1,0,0],[.5,.5,.5],[.5,-.5,.5],[0,0,1]] -> layout (Cin, K, j, kh, Co).
  - attention (2 samples ahead): pooled sums via ACT accumulator, logits via
    a GPSIMD partition all-reduce of att_w * pooled, sigmoid on ACT.  No
    PSUM used, so the conv owns all 8 banks.
  - expert mixing (1 sample ahead): the 4 products att_k * w_k run on ACT
    as activation-copies with per-partition scale; DVE folds them with an
    in-place add chain (bf16 2x mode).
  - input transform (1 sample ahead): D_j = B^T-combo of the parity planes,
    4 DVE tensor_tensor ops per ci-block (bf16 2x).
  - GEMM: per (co-block, 14-row chunk): 4 j-planes x (3 kh x 2 ci-blocks)
    bf16 matmuls accumulate N=392 columns into bank-aligned PSUM planes
    (two 4-bank chunk tiles ping-pong).
  - eviction: one GPSIMD copy per chunk moves the 4 planes to SBUF as bf16;
    DVE collapses them (Ye = M0+M1+M2, Yo = M1-M2-M3) into parity output
    planes which DMA out as bf16; the host interleaves parities and
    upcasts to f32.

Engine-queue emission order is chosen so every engine's in-order queue sees
work in the order it becomes ready (PAR ahead of evictions on GPSIMD, mix
products ahead of nothing on ACT, D/mix ahead of collapse on DVE), keeping
the PE fed back-to-back.
"""

import sys

if "/opt/trn_rl_repo" not in sys.path:
    sys.path.insert(0, "/opt/trn_rl_repo")

import numpy as np

B_TOTAL = 32
N_CORES = 8
BPC = B_TOTAL // N_CORES  # 4
CI = 256
CO = 256
K = 4
H = W = 56
PH = 58                   # padded rows
TWP = 29                  # parity-plane cols (58/2)
TW = 28                   # output tiles per row (W/2)
NJ = 4                    # wino points
NDR = 3                   # kh taps
XF = 2 * PH * TWP         # 3364 x elems per channel (par, h, twp)
WF = NJ * NDR * CO        # 3072 wino weight elems per (k, ci): (j, dr, co)
DF = NJ * PH * TW         # 6496 D elems per channel (j, h, tw)
CHR = 14                  # oh rows per PSUM chunk
NCH = H // CHR            # 4 chunks
NC_ = CHR * TW            # 392 cols per chunk (<= 512 psum bank)
OF = H * TW               # 1568 out elems per parity per co-block

_cache = {}


def _build_nc():
    from contextlib import ExitStack

    import concourse.bacc as bacc
    import concourse.bass_isa as bass_isa
    import concourse.mybir as mybir
    import concourse.tile as tile

    f32 = mybir.dt.float32
    bf16 = mybir.dt.bfloat16
    AF = mybir.ActivationFunctionType
    ALU = mybir.AluOpType
    AX = mybir.AxisListType

    nc = bacc.Bacc("TRN2", target_bir_lowering=False, debug=False)
    x_p = nc.declare_dram_parameter("x", [BPC, CI, XF], bf16, isOutput=False)
    w_p = nc.declare_dram_parameter("w", [CI, K, WF], bf16, isOutput=False)
    aw_p = nc.declare_dram_parameter("aw", [CI, K], f32, isOutput=False)
    o_p = nc.declare_dram_parameter("out", [BPC, 2, CO, OF], bf16, isOutput=True)

    with ExitStack() as ctx:
        tc = ctx.enter_context(tile.TileContext(nc))
        pw = ctx.enter_context(tc.tile_pool(name="wpool", bufs=1))
        px = ctx.enter_context(tc.tile_pool(name="xpool", bufs=2))
        pd = ctx.enter_context(tc.tile_pool(name="dpool", bufs=2))
        pagg = ctx.enter_context(tc.tile_pool(name="aggpool", bufs=2))
        pu = ctx.enter_context(tc.tile_pool(name="mixu", bufs=3))
        put1 = ctx.enter_context(tc.tile_pool(name="mixt1", bufs=2))
        pm = ctx.enter_context(tc.tile_pool(name="mpool", bufs=3))
        psd = ctx.enter_context(tc.tile_pool(name="sdpool", bufs=3))
        py = ctx.enter_context(tc.tile_pool(name="ypool", bufs=1))
        psml = ctx.enter_context(tc.tile_pool(name="small", bufs=3))
        pdump = ctx.enter_context(tc.tile_pool(name="dump", bufs=1))
        pps = ctx.enter_context(tc.tile_pool(name="cpsum", bufs=2, space="PSUM"))

        # Resident replicated weights, loaded per (ci-block, expert) so the
        # first mixing products can start before the whole bank lands.
        # DMA-queue order interleaves with the x(0) load (emitted by the
        # pipeline below before load_weights is called).
        aw_sb = [
            pw.tile([128, K], f32, tag=f"aw{c}", name=f"aw{c}") for c in range(2)
        ]
        w_sb = [
            pw.tile([128, K * WF], bf16, tag=f"w{c}", name=f"wt{c}")
            for c in range(2)
        ]

        def load_weights():
            for c in range(2):
                nc.sync.dma_start(
                    out=aw_sb[c][:, :], in_=aw_p[c * 128 : (c + 1) * 128, :]
                )
            # k-major so sample 0's mixing chases the arrivals; c1 first
            # within each k because c1's products are the cheap DVE
            # tensor_scalar path that gates the (c1-first) first conv chunk.
            for k, c in [(0, 1), (0, 0), (1, 1), (1, 0), (2, 1), (2, 0), (3, 1), (3, 0)]:
                nc.sync.dma_start(
                    out=w_sb[c][:, k * WF : (k + 1) * WF],
                    in_=w_p[c * 128 : (c + 1) * 128, k, :],
                )

        xts = {}
        atts = {}
        aggs = {}
        dts = {}
        pools = {}

        def stage_load(b):
            xb = []
            for c in range(2):
                xt = px.tile([128, XF], bf16, tag=f"x{c}")
                nc.sync.dma_start(
                    out=xt[:, :], in_=x_p[b, c * 128 : (c + 1) * 128, :]
                )
                xb.append(xt)
            xts[b] = xb

        def stage_pool(b, head=False):
            """Pooled channel sums on ACT (accumulator), emitted right after
            the x DMA so it runs as the data lands.  head=True (sample 0)
            puts ci-block 1 on DVE instead so the two halves run in
            parallel and the sigmoid unblocks sooner."""
            xb = xts[b]
            pl = []
            for c in range(2):
                pc = psml.tile([128, 1], f32, tag=f"pooled{c}")
                if head and c == 1:
                    nc.vector.tensor_reduce(
                        pc[:, :], xb[c][:, :], axis=AX.X, op=ALU.add
                    )
                else:
                    dump = pdump.tile([128, XF], bf16, tag="pooldump")
                    nc.scalar.activation(
                        dump[:, :], xb[c][:, :], AF.Copy, accum_out=pc[:, :]
                    )
                pl.append(pc)
            pools[b] = pl

        def stage_att_reduce(b):
            """pooled -> logits via GPSIMD partition all-reduce -> sigmoid."""
            pooled = pools.pop(b)
            tka = psml.tile([128, K], f32, tag="tka")
            tkb = psml.tile([128, K], f32, tag="tkb")
            nc.vector.tensor_scalar_mul(tka[:, :], aw_sb[0][:, :], pooled[0][:, :])
            nc.vector.tensor_scalar_mul(tkb[:, :], aw_sb[1][:, :], pooled[1][:, :])
            nc.vector.tensor_tensor(tka[:, :], tka[:, :], tkb[:, :], ALU.add)
            logit = psml.tile([128, K], f32, tag="logit")
            nc.gpsimd.partition_all_reduce(
                logit[:, :], tka[:, :], 128, bass_isa.ReduceOp.add
            )
            att = psml.tile([128, K], f32, tag="att")
            nc.scalar.activation(
                att[:, :], logit[:, :], AF.Sigmoid, scale=1.0 / (H * W)
            )
            atts[b] = att

        def d_ops(xt, c, eng):
            """Emit the 4 B^T-combo ops for ci-block c of one sample."""
            x3 = xt[:, :].rearrange("p (q h t) -> p q h t", q=2, h=PH)
            xe0 = x3[:, 0, :, 0:TW]
            xe1 = x3[:, 0, :, 1 : TW + 1]
            xo0 = x3[:, 1, :, 0:TW]
            xo1 = x3[:, 1, :, 1 : TW + 1]
            dt = pd.tile([128, DF], bf16, tag=f"d{c}", name=f"dt{c}")
            d3 = dt[:, :].rearrange("p (j h t) -> p j h t", j=NJ, h=PH)
            eng.tensor_tensor(d3[:, 0], xe0, xe1, ALU.subtract)
            eng.tensor_tensor(d3[:, 1], xo0, xe1, ALU.add)
            eng.tensor_tensor(d3[:, 2], xe1, xo0, ALU.subtract)
            eng.tensor_tensor(d3[:, 3], xo0, xo1, ALU.subtract)
            return dt

        def stage_d(b):
            """Input transform: D_j from parity planes, 4 TT per ci-block."""
            xb = xts.pop(b)
            dts[b] = [d_ops(xb[c], c, nc.vector) for c in range(2)]

        def stage_mix(b, head=False):
            """agg_c = sum_k att_k * w_k.

            Steady state (head=False): ci-block 0's four products on ACT;
            ci-block 1 takes k0/k1 as DVE tensor_scalar (4x mode), k2/k3 on
            ACT.  DVE folds with in-place adds (bf16 2x) in readiness order.

            head=True (sample 0): the per-k weight DMAs gate everything, so
            only c0/k0..k2 go to ACT; the DMA-critical tail (k3c0 and all of
            c1) runs as DVE tensor_scalar, which is cheap enough to chase
            each arriving weight slice.
            """
            att = atts.pop(b)
            TT = nc.vector.tensor_tensor
            TS = nc.vector.tensor_scalar_mul

            def wslice(c, k):
                return w_sb[c][:, k * WF : (k + 1) * WF]

            def act_prod(dst, c, k):
                nc.scalar.activation(
                    dst[:, :], wslice(c, k), AF.Copy, scale=att[:, k : k + 1]
                )

            ag0 = pagg.tile([128, WF], bf16, tag="agg0")
            ag1 = pagg.tile([128, WF], bf16, tag="agg1")
            if head:
                u1c0 = pu.tile([128, WF], bf16, tag="u")
                u2c0 = pu.tile([128, WF], bf16, tag="u")
                act_prod(ag0, 0, 0)
                act_prod(u1c0, 0, 1)
                act_prod(u2c0, 0, 2)
                ta = put1.tile([128, WF], bf16, tag="t1")
                tb = put1.tile([128, WF], bf16, tag="t1")
                tc_ = put1.tile([128, WF], bf16, tag="t1")
                td = put1.tile([128, WF], bf16, tag="t1")
                TS(ag1[:, :], wslice(1, 0), att[:, 0:1])
                TS(ta[:, :], wslice(1, 1), att[:, 1:2])
                TT(ag1[:, :], ag1[:, :], ta[:, :], ALU.add)
                TT(ag0[:, :], ag0[:, :], u1c0[:, :], ALU.add)
                TS(tb[:, :], wslice(1, 2), att[:, 2:3])
                TT(ag1[:, :], ag1[:, :], tb[:, :], ALU.add)
                TS(td[:, :], wslice(1, 3), att[:, 3:4])
                TT(ag1[:, :], ag1[:, :], td[:, :], ALU.add)
                TT(ag0[:, :], ag0[:, :], u2c0[:, :], ALU.add)
                TS(tc_[:, :], wslice(0, 3), att[:, 3:4])
                TT(ag0[:, :], ag0[:, :], tc_[:, :], ALU.add)
            else:
                u1c0 = pu.tile([128, WF], bf16, tag="u")
                u2c0 = pu.tile([128, WF], bf16, tag="u")
                u2c1 = pu.tile([128, WF], bf16, tag="u")
                u3c0 = pu.tile([128, WF], bf16, tag="u")
                u3c1 = pu.tile([128, WF], bf16, tag="u")
                act_prod(ag0, 0, 0)
                act_prod(u1c0, 0, 1)
                act_prod(u2c0, 0, 2)
                act_prod(u2c1, 1, 2)
                act_prod(u3c0, 0, 3)
                act_prod(u3c1, 1, 3)
                t1 = put1.tile([128, WF], bf16, tag="t1")
                TS(ag1[:, :], wslice(1, 0), att[:, 0:1])
                TS(t1[:, :], wslice(1, 1), att[:, 1:2])
                TT(ag1[:, :], ag1[:, :], t1[:, :], ALU.add)
                TT(ag0[:, :], ag0[:, :], u1c0[:, :], ALU.add)
                TT(ag0[:, :], ag0[:, :], u2c0[:, :], ALU.add)
                TT(ag1[:, :], ag1[:, :], u2c1[:, :], ALU.add)
                TT(ag0[:, :], ag0[:, :], u3c0[:, :], ALU.add)
                TT(ag1[:, :], ag1[:, :], u3c1[:, :], ALU.add)
            aggs[b] = [ag0, ag1]

        def stage_conv(b):
            """GEMM chunks with interleaved eviction (GPSIMD) and collapse
            (DVE), then the output DMAs."""
            db = dts.pop(b)
            ab = aggs.pop(b)
            d3s = [
                dt[:, :].rearrange("p (j h t) -> p j h t", j=NJ, h=PH) for dt in db
            ]
            for cb in range(2):
                yt = py.tile([128, 2 * OF], bf16, tag="y")
                y4 = yt[:, :].rearrange("p (q h t) -> p q h t", q=2, h=H)
                for ch in range(NCH):
                    r0 = ch * CHR
                    ps = pps.tile([128, NJ * 512], f32, tag="convps")
                    # two passes over the j-groups (c1 then c0) so the PE
                    # has 12 matmuls of runway before agg(c0) is needed
                    # (matters for the first sample, harmless after)
                    for c in (1, 0):
                        for j in range(NJ):
                            out3 = ps[:, j * 512 : j * 512 + NC_].rearrange(
                                "p (h t) -> p h t", h=CHR
                            )
                            for dr in range(NDR):
                                base = (j * NDR + dr) * CO + cb * 128
                                nc.tensor.matmul(
                                    out3[:, :, :],
                                    lhsT=ab[c][:, base : base + 128],
                                    rhs=d3s[c][:, j, r0 + dr : r0 + dr + CHR, :],
                                    start=(c == 1 and dr == 0),
                                    stop=(c == 0 and dr == NDR - 1),
                                )
                    mt = pm.tile([128, NJ * NC_], bf16, tag="m")
                    nc.gpsimd.tensor_copy(
                        mt[:, :].rearrange("p (j n) -> p j n", j=NJ),
                        ps[:, :].rearrange("p (j n) -> p j n", j=NJ)[:, :, 0:NC_],
                    )
                    # collapse this chunk on DVE
                    m3 = mt[:, :].rearrange("p (j h t) -> p j h t", j=NJ, h=CHR)
                    s = psd.tile([128, NC_], bf16, tag="s")
                    d = psd.tile([128, NC_], bf16, tag="d")
                    s3 = s[:, :].rearrange("p (h t) -> p h t", h=CHR)
                    d3 = d[:, :].rearrange("p (h t) -> p h t", h=CHR)
                    # s/d on GPSIMD (right after its eviction) frees DVE slack
                    nc.gpsimd.tensor_tensor(s3, m3[:, 1], m3[:, 2], ALU.add)
                    nc.gpsimd.tensor_tensor(d3, m3[:, 1], m3[:, 2], ALU.subtract)
                    nc.vector.tensor_tensor(
                        y4[:, 0, r0 : r0 + CHR, :], m3[:, 0], s3, ALU.add
                    )
                    nc.vector.tensor_tensor(
                        y4[:, 1, r0 : r0 + CHR, :], d3, m3[:, 3], ALU.subtract
                    )
                    if ch % 2 == 1:
                        # flush the finished half so the tail DMA is short
                        hf = ch // 2
                        half = OF // 2
                        for par in range(2):
                            nc.sync.dma_start(
                                out=o_p[
                                    b,
                                    par,
                                    cb * 128 : (cb + 1) * 128,
                                    hf * half : (hf + 1) * half,
                                ],
                                in_=yt[:, :].rearrange("p (q f) -> p q f", q=2)[
                                    :, par, hf * half : (hf + 1) * half
                                ],
                            )

        # ---- software pipeline ----
        # Preamble (sample 0): x(0) heads the DMA queue, then the per-k
        # weight loads.  A dummy sigmoid at t=0 preloads the act-function
        # table off the critical path; pooling is split ACT/DVE; the D
        # transform is split DVE/GPSIMD; warm-up matmuls gated on an early
        # weight slice keep the PE p-state at full clock by the time the
        # first real matmul issues.
        dsig = psml.tile([128, 2], f32, tag="dsig")
        nc.vector.memset(dsig[:, :], 0.0)
        nc.scalar.activation(dsig[:, :], dsig[:, :], AF.Sigmoid)
        stage_load(0)
        load_weights()
        # pooled: c0 on ACT (accumulator), c1 on DVE
        pc0 = psml.tile([128, 1], f32, tag="pooled0")
        dump = pdump.tile([128, XF], bf16, tag="pooldump")
        nc.scalar.activation(dump[:, :], xts[0][0][:, :], AF.Copy, accum_out=pc0[:, :])
        dt_c0 = d_ops(xts[0][0], 0, nc.vector)
        pc1 = psml.tile([128, 1], f32, tag="pooled1")
        nc.vector.tensor_reduce(pc1[:, :], xts[0][1][:, :], axis=AX.X, op=ALU.add)
        pools[0] = [pc0, pc1]
        stage_att_reduce(0)
        dt_c1 = d_ops(xts[0][1], 1, nc.gpsimd)
        dts[0] = [dt_c0, dt_c1]
        del xts[0]
        # PE p-state warm-up: gated on the first-arriving weight slice
        # (~10us), sized to end just before the first real matmuls.
        warm_ps = pps.tile([128, NJ * 512], f32, tag="convps")
        for _ in range(62):
            nc.tensor.matmul(
                warm_ps[:, 0:512],
                lhsT=w_sb[1][:, 0:128],
                rhs=w_sb[1][:, 0:512],
                start=True,
                stop=True,
            )
        stage_mix(0, head=True)
        stage_load(1)
        stage_pool(1)
        stage_att_reduce(1)
        for b in range(BPC):
            # Window b queues: DVE: D(b+1), mix(b+1) TS+folds, collapse(b);
            # ACT: products(b+1), pooled(b+2), sigmoid(b+2) (emitted after
            # conv so GPSIMD's PAR(b+2) queues behind the evictions(b));
            # GPSIMD: evictions(b), then PAR(b+2).
            if b + 1 < BPC:
                stage_d(b + 1)
                stage_mix(b + 1)
            if b + 2 < BPC:
                stage_load(b + 2)
                stage_pool(b + 2)
            stage_conv(b)
            if b + 2 < BPC:
                stage_att_reduce(b + 2)

    nc.compile()
    return nc


def _get_nc():
    if "nc" not in _cache:
        _cache["nc"] = _build_nc()
    return _cache["nc"]


def _make_in_maps(x, att_w, weight):
    from ml_dtypes import bfloat16

    x = np.asarray(x, dtype=np.float32)
    att_w = np.asarray(att_w, dtype=np.float32)
    weight = np.asarray(weight, dtype=np.float32)
    # pad to (58, 58), split w-parity, cast bf16: (B, CI, 2, 58, 29)
    xp = np.pad(x, ((0, 0), (0, 0), (1, 1), (1, 1)))
    xh = np.empty((B_TOTAL, CI, 2, PH, TWP), dtype=bfloat16)
    xh[:, :, 0] = xp[:, :, :, 0::2]
    xh[:, :, 1] = xp[:, :, :, 1::2]
    xh = xh.reshape(B_TOTAL, CI, XF)
    # wino expert banks: (K, Cout, Cin, kh, kw) -> (Cin, K, j, kh, Cout)
    G = np.array(
        [[1, 0, 0], [0.5, 0.5, 0.5], [0.5, -0.5, 0.5], [0, 0, 1]], np.float32
    )
    wj = np.einsum("jd,koihd->ikjho", G, weight)
    wj = np.ascontiguousarray(wj).astype(bfloat16).reshape(CI, K, WF)
    awt = np.ascontiguousarray(att_w.T)  # (CI, K) f32
    return [
        {
            "x": np.ascontiguousarray(xh[i * BPC : (i + 1) * BPC]),
            "w": wj,
            "aw": awt,
        }
        for i in range(N_CORES)
    ]


def _run(x, att_w, weight, trace=False, **spmd_kwargs):
    from concourse.bass_utils import run_bass_kernel_spmd

    nc = _get_nc()
    in_maps = _make_in_maps(x, att_w, weight)
    res = run_bass_kernel_spmd(
        nc, in_maps, list(range(N_CORES)), trace=trace, **spmd_kwargs
    )
    o = np.concatenate([r["out"] for r in res.results], axis=0)
    # (B, 2, CO, H*TW) bf16 -> interleave parities, upcast
    o = o.reshape(B_TOTAL, 2, CO, H, TW).astype(np.float32)
    out = np.empty((B_TOTAL, CO, H, W), dtype=np.float32)
    out[:, :, :, 0::2] = o[:, 0]
    out[:, :, :, 1::2] = o[:, 1]
    return out, res


def kernel(x, att_w, weight):
    out, _ = _run(x, att_w, weight)
    return out
